# revision 15
# baseline (speedup 1.0000x reference)
"""CapsLayer2D dynamic-routing kernel for 8x TRN2 NeuronCores — v4.

Shapes (hardcoded):
  inputs: [B=16, R=8, C=8, I=128, DIN=16] fp32
  W:      [K=32, I=128, DIN=16, DOUT=16] fp32
  out:    [B, R, C, K, DOUT] fp32

Math: 3-round dynamic routing, closed form (verified 6e-6 vs reference):
  U[p,k] = res (I x O);  s0 = mean_i U_i;  A = U^T U
  y1 = A s0 = U^T(U s0) ; y2 = A y1
  g = factor(s0); s1 = s0 + g*y1; f = factor(s1)
  out = factor(s2)*s2,  s2 = s0 + (g+f)*y1 + f*g*y2
  factor(s) = (|s|^2/(1+|s|^2)) / sqrt(|s|^2+eps)

Per-core plan (batch sharded across 8 cores, W replicated):
  Host pre-builds Xt[(i%4)*32+d, (i//4)*128+p] and W_r[(i%4)*32+d,
  (i//4)*512+k*16+o] fp16, d padded 16->32 (matmul operands need
  32-aligned partition starts). PE: 32 accumulating matmuls -> s0
  (overlapped with the W DMA); 512 banded 32-deep 128-col matmuls ->
  res fp16 in (g, i, k8, o) order (g = k-group of 8); Scalar engine
  evacuates PSUM. All tiles coexist (no pool reuse), so routing has no
  WAR barrier against production and starts on group 0 immediately.
  Routing per (round, group) in i-halves of 64:
    uv-mul (2x) -> o-tree adds 16->8->4->2 (2x) -> q2-direct (dup pair)
    ut-mul via (oh=8, ol=2) pair views (2x) -> i-tree: adjacent-pair
    level then halving adds, per half; halves summed into y.
  Scratch: A[16KB] mul buffer, B[8KB] first tree level, trees ping-pong
  into dead regions of A. Squash factors g/f/h off the sweep path.
"""

import sys

import numpy as np

sys.path.insert(0, "/opt/trn_rl_repo")

P, I, D, D2, K, O = 128, 128, 16, 32, 32, 16
KC = 8          # k-group size
NG = K // KC    # 4 groups
GN = I * KC * O  # 16384 elements per group block
KO = K * O      # 512
HB = 8192       # half-group block (64 i's)
N_CORES = 8
EPS = 1e-7

_PROGRAM = None


def _build_program():
    from contextlib import ExitStack

    import concourse.tile as tile
    from concourse import bacc, mybir

    F32 = mybir.dt.float32
    F16 = mybir.dt.float16
    ADD = mybir.AluOpType.add
    X = mybir.AxisListType.X
    SQRT = mybir.ActivationFunctionType.Sqrt

    nc = bacc.Bacc("TRN2", target_bir_lowering=False, debug=False)

    xt_d = nc.dram_tensor("xt", [P, 32 * 128], F16, kind="ExternalInput").ap()
    wr_d = nc.dram_tensor("wr", [P, 32 * KO], F16, kind="ExternalInput").ap()
    out_d = nc.dram_tensor("out", [P, KO], F32, kind="ExternalOutput").ap()

    with ExitStack() as ctx:
        tc = ctx.enter_context(tile.TileContext(nc))

        pp = ctx.enter_context(tc.tile_pool(name="pp", bufs=2, space="PSUM"))
        rp = ctx.enter_context(tc.tile_pool(name="resp", bufs=1))
        sm = ctx.enter_context(tc.tile_pool(name="small", bufs=1))

        res = rp.tile([P, NG * GN], F16)     # [P, 65536] (g, i, k8, o)
        A = rp.tile([P, HB], F16)            # mul outputs (uv / ut), half
        B = rp.tile([P, 4096], F16)          # first tree level
        q2 = rp.tile([P, I * KC * 2], F16)   # [(i,k8), 2] dup'd uv result
        Xt = rp.tile([P, 32 * 128], F16)
        Wr = rp.tile([P, 32 * KO], F16)

        # ---- small tiles ----
        s0h = sm.tile([P, KO], F16, tag="s0h")
        y1h = sm.tile([P, KO], F16, tag="y1h")
        y2h = sm.tile([P, KO], F16, tag="y2h")
        sqb = sm.tile([P, KO], F32, tag="sqb")
        s2f = sm.tile([P, KO], F32, tag="s2f")
        ytmp = sm.tile([P, KC * O], F16, tag="ytmp")
        eps_t = sm.tile([P, 1], F32, tag="eps")
        nc.vector.memset(eps_t[:], EPS)

        def factor(src, out32, tag):
            """out32[p, K] = (nsq/(1+nsq))/sqrt(nsq+eps), nsq over o."""
            nc.scalar.square(sqb[:], src)
            nsq = sm.tile([P, K], F32, tag=f"nsq_{tag}")
            nc.vector.tensor_reduce(
                nsq[:], sqb[:].rearrange("p (k o) -> p k o", k=K), X, ADD
            )
            rt = sm.tile([P, K], F32, tag="f_rt")
            nc.scalar.activation(rt[:], nsq[:], SQRT, bias=eps_t[:])
            b1 = sm.tile([P, K], F32, tag="f_b1")
            nc.scalar.add(b1[:], nsq[:], 1.0)
            den = sm.tile([P, K], F32, tag="f_den")
            nc.vector.tensor_mul(den[:], rt[:], b1[:])
            rin = sm.tile([P, K], F32, tag="f_rin")
            nc.vector.reciprocal(rin[:], den[:])
            nc.vector.tensor_mul(out32[:], nsq[:], rin[:])

        def bcast_o(v32):
            return v32[:].unsqueeze(2).broadcast_to([P, K, O])

        # ---- input DMAs (chunked; s0 matmuls chase the Wr pieces) ----
        for q in range(2):
            nc.sync.dma_start(
                Xt[:, q * 2048:(q + 1) * 2048], xt_d[:, q * 2048:(q + 1) * 2048]
            )
        for q in range(8):
            nc.sync.dma_start(
                Wr[:, q * 2048:(q + 1) * 2048], wr_d[:, q * 2048:(q + 1) * 2048]
            )

        # ---- PE warm-up: dummy matmuls on the first Xt piece keep the
        # HAM activity window busy during the Wr DMA so the s0 chain and
        # production run at the warm (2.4 GHz) clock.
        q0 = pp.tile([P, 2048], F32, tag="quad")
        for w in range(24):
            nc.tensor.matmul(
                q0[:, 1024:1152],
                Xt[0:32, 0:128],
                Xt[0:32, 0:128],
                start=(w == 0),
                stop=(w == 23),
                tile_position=(0, 0),
            )

        # ---- s0 = X W / I : 32 accumulating full-depth matmuls ----
        for c in range(32):
            nc.tensor.matmul(
                q0[:, 0:KO],
                Xt[:, c * 128:(c + 1) * 128],
                Wr[:, c * KO:(c + 1) * KO],
                start=(c == 0),
                stop=(c == 31),
            )
        nc.scalar.activation(
            s0h[:], q0[:, 0:KO],
            mybir.ActivationFunctionType.Copy, scale=1.0 / I,
        )
        g32 = sm.tile([P, K], F32, tag="g32")

        # ---- res production: (g, i, k8, o) order, g-outer ----
        # quad (g, cq): 16 matmuls (band b, chunk 4cq+j) at psum col
        # b*512 + j*128 (bank b <- row-tile bank-conflict rule).
        for g in range(NG):
            for cq in range(8):
                qt = pp.tile([P, 2048], F32, tag="quad")
                for b in range(4):
                    r0 = b * 32
                    for j in range(4):
                        c = 4 * cq + j
                        nc.tensor.matmul(
                            qt[:, b * 512 + j * 128:b * 512 + (j + 1) * 128],
                            Xt[r0:r0 + 32, c * 128:(c + 1) * 128],
                            Wr[r0:r0 + 32, c * KO + g * 128:c * KO + (g + 1) * 128],
                            start=True,
                            stop=True,
                            tile_position=(r0, 0),
                        )
                # evac: psum (b, j, ko) -> res cols base + j*512 + b*128
                base = g * GN + cq * 2048
                dst = (
                    res[:, base:base + 2048]
                    .rearrange("p (j b o) -> p j b o", j=4, b=4)
                    .transpose([0, 2, 1, 3])
                )
                src = qt[:].rearrange("p (b j o) -> p b j o", b=4, j=4)
                with nc.allow_low_precision(reason="res fp16"):
                    # first quads alternate onto the (still idle) Vector
                    # engine so group 0 is ready sooner; the rest stay on
                    # Scalar to keep the Vector queue clear for routing.
                    if g == 0 and cq in (1, 3):
                        nc.vector.tensor_copy(dst, src)
                    else:
                        nc.scalar.copy(dst, src)

        # ---- routing ----
        def half_round(v_h16, y_out16):
            """y = U^T (U v) per group, in i-halves of 64.

            Scratch walk per (g, hf):
              uv  -> A[0:8192]
              t8  -> B[0:4096]   t4 -> A[0:2048]   t2 -> A[2048:3072]
              q2h -> q2[hf]
              ut  -> A[0:8192]
              L1  -> B[0:4096]   L2 -> A[0:2048]   L3 -> A[2048:3072]
              L4  -> A[3072:3584]  L5 -> A[3584:3840]  L6 -> y half
            """
            for g in range(NG):
                rg = res[:, g * GN:(g + 1) * GN]
                vg = (
                    v_h16[:, g * KC * O:(g + 1) * KC * O]
                    .rearrange("p (k o) -> p k o", k=KC)
                )
                ys = y_out16[:, g * KC * O:(g + 1) * KC * O]
                for hf in range(2):
                    rh = rg[:, hf * HB:(hf + 1) * HB]
                    # uv mul
                    av = A[:].rearrange("p (i k o) -> p i k o", i=64, k=KC, o=O)
                    nc.vector.tensor_mul(
                        av,
                        rh.rearrange("p (i k o) -> p i k o", i=64, k=KC, o=O),
                        vg.unsqueeze(1).broadcast_to([P, 64, KC, O]),
                    )
                    # o-tree 16->8->4->2
                    tv = A[:].rearrange("p (ik o) -> p ik o", o=16)
                    t8 = B[:].rearrange("p (ik o) -> p ik o", o=8)
                    nc.vector.tensor_add(t8, tv[:, :, 0:8], tv[:, :, 8:16])
                    t4 = A[:, 0:2048].rearrange("p (ik o) -> p ik o", o=4)
                    nc.vector.tensor_add(t4, t8[:, :, 0:4], t8[:, :, 4:8])
                    t2 = A[:, 2048:3072].rearrange("p (ik o) -> p ik o", o=2)
                    nc.vector.tensor_add(t2, t4[:, :, 0:2], t4[:, :, 2:4])
                    # q2[ik, j] = t2[ik,0] + t2[ik,1]
                    qh = q2[:, hf * 1024:(hf + 1) * 1024].rearrange(
                        "p (ik j) -> p ik j", j=2
                    )
                    nc.vector.tensor_add(
                        qh,
                        t2[:, :, 0:1].broadcast_to([P, 64 * KC, 2]),
                        t2[:, :, 1:2].broadcast_to([P, 64 * KC, 2]),
                    )
                    # ut mul (pair view oh=8, ol=2)
                    am = A[:].rearrange("p (ik oh ol) -> p ik oh ol", oh=8, ol=2)
                    nc.vector.tensor_mul(
                        am,
                        rh.rearrange("p (ik oh ol) -> p ik oh ol", oh=8, ol=2),
                        qh.unsqueeze(2).broadcast_to([P, 64 * KC, 8, 2]),
                    )
                    # i-tree: adjacent-pair L1, then halving in A
                    a2 = A[:].rearrange("p (i2 pr r) -> p i2 pr r", i2=32, pr=2)
                    l1 = B[:].rearrange("p (i r) -> p i r", i=32)
                    nc.vector.tensor_add(l1, a2[:, :, 0], a2[:, :, 1])
                    spots = [(0, 2048), (2048, 1024), (3072, 512), (3584, 256)]
                    cur, size = B[:], 32
                    for lv in range(5):
                        half = size // 2
                        cv = cur.rearrange("p (i r) -> p i r", i=size)
                        if lv < 4:
                            off, ln = spots[lv]
                            nxt = A[:, off:off + ln]
                        elif hf == 0:
                            nxt = ys
                        else:
                            nxt = ytmp[:]
                        nv = nxt.rearrange("p (i r) -> p i r", i=half)
                        nc.vector.tensor_add(nv, cv[:, 0:half], cv[:, half:size])
                        cur, size = nxt, half
                # y = yA + yB
                nc.vector.tensor_add(ys, ys, ytmp[:])

        with nc.allow_low_precision(reason="fp16 routing"):
            # round 1: y1 = A s0
            half_round(s0h, y1h)
            # g = factor(s0) (kept off the routing-start critical path)
            factor(s0h[:], g32, "g")
            # f = factor(s1), s1 = s0 + g*y1 (overlaps round 2)
            outf = sm.tile([P, KO], F32, tag="outf")
            s1f = outf
            nc.vector.tensor_mul(
                s1f[:].rearrange("p (k o) -> p k o", k=K),
                y1h[:].rearrange("p (k o) -> p k o", k=K),
                bcast_o(g32),
            )
            nc.vector.tensor_add(s1f[:], s1f[:], s0h[:])
            # round 2: y2 = A y1
            half_round(y1h, y2h)

            f32_ = sm.tile([P, K], F32, tag="f32_")
            factor(s1f[:], f32_, "f")
            # s2 = s0 + (g+f)*y1 + (f*g)*y2
            gf = sm.tile([P, K], F32, tag="gf")
            nc.vector.tensor_add(gf[:], g32[:], f32_[:])
            fg = sm.tile([P, K], F32, tag="fg")
            nc.vector.tensor_mul(fg[:], f32_[:], g32[:])
            nc.vector.tensor_mul(
                s2f[:].rearrange("p (k o) -> p k o", k=K),
                y1h[:].rearrange("p (k o) -> p k o", k=K),
                bcast_o(gf),
            )
            nc.vector.tensor_add(s2f[:], s2f[:], s0h[:])
            nc.vector.tensor_mul(
                sqb[:].rearrange("p (k o) -> p k o", k=K),
                y2h[:].rearrange("p (k o) -> p k o", k=K),
                bcast_o(fg),
            )
            nc.vector.tensor_add(s2f[:], s2f[:], sqb[:])
            # out = factor(s2) * s2
            h32 = sm.tile([P, K], F32, tag="h32")
            factor(s2f[:], h32, "h")
            nc.vector.tensor_mul(
                outf[:].rearrange("p (k o) -> p k o", k=K),
                s2f[:].rearrange("p (k o) -> p k o", k=K),
                bcast_o(h32),
            )
        nc.sync.dma_start(out_d, outf[:])

    nc.compile()
    return nc


def _host_prep(x, W):
    """x: [B,R,C,I,D] f32; W: [K,I,D,O] f32 -> per-core Xt + shared W_r.

    Xt[(i%4)*32+d, (i//4)*128+p] = x[p, i, d] (d < 16, pad to 32).
    W_r[(i%4)*32+d, (i//4)*512+k*16+o] = W[k, i, d, o].
    """
    xs = x.reshape(N_CORES, P, I, D)
    a = xs.transpose(0, 2, 3, 1).reshape(N_CORES, 32, 4, D, P)
    ap = np.zeros((N_CORES, 32, 4, D2, P), np.float32)
    ap[:, :, :, 0:D, :] = a
    xt = (
        ap.transpose(0, 2, 3, 1, 4)
        .reshape(N_CORES, 128, 32 * 128)
        .astype(np.float16)
    )
    b = W.transpose(1, 2, 0, 3).reshape(32, 4, D, KO)
    bp = np.zeros((32, 4, D2, KO), np.float32)
    bp[:, :, 0:D, :] = b
    wr = bp.transpose(1, 2, 0, 3).reshape(128, 32 * KO).astype(np.float16)
    return xt, wr


def _get_program():
    global _PROGRAM
    if _PROGRAM is None:
        _PROGRAM = _build_program()
    return _PROGRAM


def kernel(**inputs):
    x = np.ascontiguousarray(np.asarray(inputs["inputs"], dtype=np.float32))
    W = np.ascontiguousarray(np.asarray(inputs["W"], dtype=np.float32))
    assert x.shape == (16, 8, 8, 128, 16) and W.shape == (32, 128, 16, 16)

    from concourse.bass_utils import run_bass_kernel_spmd

    nc = _get_program()
    xt, wr = _host_prep(x, W)
    in_maps = [
        {"xt": np.ascontiguousarray(xt[c]), "wr": wr} for c in range(N_CORES)
    ]
    r = run_bass_kernel_spmd(nc, in_maps, list(range(N_CORES)))
    outs = [r.results[c]["out"].reshape(2, 8, 8, K, O) for c in range(N_CORES)]
    return np.concatenate(outs, axis=0).astype(np.float32)


# revision 16
# speedup vs baseline: 1.0012x; 1.0012x over previous
"""CapsLayer2D dynamic-routing kernel for 8x TRN2 NeuronCores — v4.

Shapes (hardcoded):
  inputs: [B=16, R=8, C=8, I=128, DIN=16] fp32
  W:      [K=32, I=128, DIN=16, DOUT=16] fp32
  out:    [B, R, C, K, DOUT] fp32

Math: 3-round dynamic routing, closed form (verified 6e-6 vs reference):
  U[p,k] = res (I x O);  s0 = mean_i U_i;  A = U^T U
  y1 = A s0 = U^T(U s0) ; y2 = A y1
  g = factor(s0); s1 = s0 + g*y1; f = factor(s1)
  out = factor(s2)*s2,  s2 = s0 + (g+f)*y1 + f*g*y2
  factor(s) = (|s|^2/(1+|s|^2)) / sqrt(|s|^2+eps)

Per-core plan (batch sharded across 8 cores, W replicated):
  Host pre-builds Xt[(i%4)*32+d, (i//4)*128+p] and W_r[(i%4)*32+d,
  (i//4)*512+k*16+o] fp16, d padded 16->32 (matmul operands need
  32-aligned partition starts). PE: 32 accumulating matmuls -> s0
  (overlapped with the W DMA); 512 banded 32-deep 128-col matmuls ->
  res fp16 in (g, i, k8, o) order (g = k-group of 8); Scalar engine
  evacuates PSUM. All tiles coexist (no pool reuse), so routing has no
  WAR barrier against production and starts on group 0 immediately.
  Routing per (round, group) in i-halves of 64:
    uv-mul (2x) -> o-tree adds 16->8->4->2 (2x) -> q2-direct (dup pair)
    ut-mul via (oh=8, ol=2) pair views (2x) -> i-tree: adjacent-pair
    level then halving adds, per half; halves summed into y.
  Scratch: A[16KB] mul buffer, B[8KB] first tree level, trees ping-pong
  into dead regions of A. Squash factors g/f/h off the sweep path.
"""

import sys

import numpy as np

sys.path.insert(0, "/opt/trn_rl_repo")

P, I, D, D2, K, O = 128, 128, 16, 32, 32, 16
KC = 8          # k-group size
NG = K // KC    # 4 groups
GN = I * KC * O  # 16384 elements per group block
KO = K * O      # 512
HB = 8192       # half-group block (64 i's)
N_CORES = 8
EPS = 1e-7

_PROGRAM = None


def _build_program():
    from contextlib import ExitStack

    import concourse.tile as tile
    from concourse import bacc, mybir

    F32 = mybir.dt.float32
    F16 = mybir.dt.float16
    ADD = mybir.AluOpType.add
    X = mybir.AxisListType.X
    SQRT = mybir.ActivationFunctionType.Sqrt

    nc = bacc.Bacc("TRN2", target_bir_lowering=False, debug=False)

    xt_d = nc.dram_tensor("xt", [P, 32 * 128], F16, kind="ExternalInput").ap()
    wr_d = nc.dram_tensor("wr", [P, 32 * KO], F16, kind="ExternalInput").ap()
    out_d = nc.dram_tensor("out", [P, KO], F32, kind="ExternalOutput").ap()

    with ExitStack() as ctx:
        tc = ctx.enter_context(tile.TileContext(nc))

        pp = ctx.enter_context(tc.tile_pool(name="pp", bufs=2, space="PSUM"))
        rp = ctx.enter_context(tc.tile_pool(name="resp", bufs=1))
        sm = ctx.enter_context(tc.tile_pool(name="small", bufs=1))

        res = rp.tile([P, NG * GN], F16)     # [P, 65536] (g, i, k8, o)
        A = rp.tile([P, HB], F16)            # mul outputs (uv / ut), half
        B = rp.tile([P, 4096], F16)          # first tree level
        q2 = rp.tile([P, I * KC * 2], F16)   # [(i,k8), 2] dup'd uv result
        Xt = rp.tile([P, 32 * 128], F16)
        Wr = rp.tile([P, 32 * KO], F16)

        # ---- small tiles ----
        s0h = sm.tile([P, KO], F16, tag="s0h")
        y1h = sm.tile([P, KO], F16, tag="y1h")
        y2h = sm.tile([P, KO], F16, tag="y2h")
        sqb = sm.tile([P, KO], F32, tag="sqb")
        s2f = sm.tile([P, KO], F32, tag="s2f")
        ytmp = sm.tile([P, KC * O], F16, tag="ytmp")
        eps_t = sm.tile([P, 1], F32, tag="eps")
        nc.vector.memset(eps_t[:], EPS)

        def factor(src, out32, tag):
            """out32[p, K] = (nsq/(1+nsq))/sqrt(nsq+eps), nsq over o."""
            nc.scalar.square(sqb[:], src)
            nsq = sm.tile([P, K], F32, tag=f"nsq_{tag}")
            nc.vector.tensor_reduce(
                nsq[:], sqb[:].rearrange("p (k o) -> p k o", k=K), X, ADD
            )
            rt = sm.tile([P, K], F32, tag="f_rt")
            nc.scalar.activation(rt[:], nsq[:], SQRT, bias=eps_t[:])
            b1 = sm.tile([P, K], F32, tag="f_b1")
            nc.scalar.add(b1[:], nsq[:], 1.0)
            den = sm.tile([P, K], F32, tag="f_den")
            nc.vector.tensor_mul(den[:], rt[:], b1[:])
            rin = sm.tile([P, K], F32, tag="f_rin")
            nc.vector.reciprocal(rin[:], den[:])
            nc.vector.tensor_mul(out32[:], nsq[:], rin[:])

        def bcast_o(v32):
            return v32[:].unsqueeze(2).broadcast_to([P, K, O])

        # ---- input DMAs (chunked; s0 matmuls chase the Wr pieces) ----
        for q in range(2):
            nc.sync.dma_start(
                Xt[:, q * 2048:(q + 1) * 2048], xt_d[:, q * 2048:(q + 1) * 2048]
            )
        for q in range(8):
            nc.sync.dma_start(
                Wr[:, q * 2048:(q + 1) * 2048], wr_d[:, q * 2048:(q + 1) * 2048]
            )

        # ---- PE warm-up: dummy matmuls on the first Xt piece keep the
        # HAM activity window busy during the Wr DMA so the s0 chain and
        # production run at the warm (2.4 GHz) clock.
        q0 = pp.tile([P, 2048], F32, tag="quad")
        for w in range(24):
            nc.tensor.matmul(
                q0[:, 1024:1152],
                Xt[0:32, 0:128],
                Xt[0:32, 0:128],
                start=(w == 0),
                stop=(w == 23),
                tile_position=(0, 0),
            )

        # ---- s0 = X W / I : 32 accumulating full-depth matmuls ----
        for c in range(32):
            nc.tensor.matmul(
                q0[:, 0:KO],
                Xt[:, c * 128:(c + 1) * 128],
                Wr[:, c * KO:(c + 1) * KO],
                start=(c == 0),
                stop=(c == 31),
            )
        nc.scalar.activation(
            s0h[:], q0[:, 0:KO],
            mybir.ActivationFunctionType.Copy, scale=1.0 / I,
        )
        g32 = sm.tile([P, K], F32, tag="g32")

        # ---- res production: (g, i, k8, o) order, g-outer ----
        # quad (g, cq): 16 matmuls (band b, chunk 4cq+j) at psum col
        # b*512 + j*128 (bank b <- row-tile bank-conflict rule).
        for g in range(NG):
            for cq in range(8):
                qt = pp.tile([P, 2048], F32, tag="quad")
                for b in range(4):
                    r0 = b * 32
                    for j in range(4):
                        c = 4 * cq + j
                        nc.tensor.matmul(
                            qt[:, b * 512 + j * 128:b * 512 + (j + 1) * 128],
                            Xt[r0:r0 + 32, c * 128:(c + 1) * 128],
                            Wr[r0:r0 + 32, c * KO + g * 128:c * KO + (g + 1) * 128],
                            start=True,
                            stop=True,
                            tile_position=(r0, 0),
                        )
                # evac: psum (b, j, ko) -> res cols base + j*512 + b*128
                base = g * GN + cq * 2048
                dst = (
                    res[:, base:base + 2048]
                    .rearrange("p (j b o) -> p j b o", j=4, b=4)
                    .transpose([0, 2, 1, 3])
                )
                src = qt[:].rearrange("p (b j o) -> p b j o", b=4, j=4)
                with nc.allow_low_precision(reason="res fp16"):
                    # first quads alternate onto the (still idle) Vector
                    # engine so group 0 is ready sooner; the rest stay on
                    # Scalar to keep the Vector queue clear for routing.
                    if g == 0 and cq in (1, 3):
                        nc.vector.tensor_copy(dst, src)
                    else:
                        nc.scalar.copy(dst, src)

        # ---- routing ----
        def half_round(v_h16, y_out16):
            """y = U^T (U v) per group, in i-halves of 64.

            Scratch walk per (g, hf):
              uv  -> A[0:8192]
              t8  -> B[0:4096]   t4 -> A[0:2048]   t2 -> A[2048:3072]
              q2h -> q2[hf]
              ut  -> A[0:8192]
              L1  -> B[0:4096]   L2 -> A[0:2048]   L3 -> A[2048:3072]
              L4  -> A[3072:3584]  L5 -> A[3584:3840]  L6 -> y half
            """
            for g in range(NG):
                rg = res[:, g * GN:(g + 1) * GN]
                vg = (
                    v_h16[:, g * KC * O:(g + 1) * KC * O]
                    .rearrange("p (k o) -> p k o", k=KC)
                )
                ys = y_out16[:, g * KC * O:(g + 1) * KC * O]
                for hf in range(2):
                    rh = rg[:, hf * HB:(hf + 1) * HB]
                    # uv mul
                    av = A[:].rearrange("p (i k o) -> p i k o", i=64, k=KC, o=O)
                    nc.vector.tensor_mul(
                        av,
                        rh.rearrange("p (i k o) -> p i k o", i=64, k=KC, o=O),
                        vg.unsqueeze(1).broadcast_to([P, 64, KC, O]),
                    )
                    # o-tree 16->8->4->2
                    tv = A[:].rearrange("p (ik o) -> p ik o", o=16)
                    t8 = B[:].rearrange("p (ik o) -> p ik o", o=8)
                    nc.vector.tensor_add(t8, tv[:, :, 0:8], tv[:, :, 8:16])
                    t4 = A[:, 0:2048].rearrange("p (ik o) -> p ik o", o=4)
                    nc.vector.tensor_add(t4, t8[:, :, 0:4], t8[:, :, 4:8])
                    t2 = A[:, 2048:3072].rearrange("p (ik o) -> p ik o", o=2)
                    nc.vector.tensor_add(t2, t4[:, :, 0:2], t4[:, :, 2:4])
                    # q2[ik, j] = t2[ik,0] + t2[ik,1]
                    qh = q2[:, hf * 1024:(hf + 1) * 1024].rearrange(
                        "p (ik j) -> p ik j", j=2
                    )
                    nc.vector.tensor_add(
                        qh,
                        t2[:, :, 0:1].broadcast_to([P, 64 * KC, 2]),
                        t2[:, :, 1:2].broadcast_to([P, 64 * KC, 2]),
                    )
                    # ut mul (pair view oh=8, ol=2)
                    am = A[:].rearrange("p (ik oh ol) -> p ik oh ol", oh=8, ol=2)
                    nc.vector.tensor_mul(
                        am,
                        rh.rearrange("p (ik oh ol) -> p ik oh ol", oh=8, ol=2),
                        qh.unsqueeze(2).broadcast_to([P, 64 * KC, 8, 2]),
                    )
                    # i-tree: adjacent-pair L1, then halving in A
                    a2 = A[:].rearrange("p (i2 pr r) -> p i2 pr r", i2=32, pr=2)
                    l1 = B[:].rearrange("p (i r) -> p i r", i=32)
                    nc.vector.tensor_add(l1, a2[:, :, 0], a2[:, :, 1])
                    spots = [(0, 2048), (2048, 1024), (3072, 512), (3584, 256)]
                    cur, size = B[:], 32
                    for lv in range(5):
                        half = size // 2
                        cv = cur.rearrange("p (i r) -> p i r", i=size)
                        if lv < 4:
                            off, ln = spots[lv]
                            nxt = A[:, off:off + ln]
                        elif hf == 0:
                            nxt = ys
                        else:
                            nxt = ytmp[:]
                        nv = nxt.rearrange("p (i r) -> p i r", i=half)
                        nc.vector.tensor_add(nv, cv[:, 0:half], cv[:, half:size])
                        cur, size = nxt, half
                # y = yA + yB
                nc.vector.tensor_add(ys, ys, ytmp[:])

        with nc.allow_low_precision(reason="fp16 routing"):
            # round 1: y1 = A s0
            half_round(s0h, y1h)
            # Scale y1 by 1/64 (exact in fp16) before round 2 so that
            # y2/64 = A(y1/64) stays far from the fp16 max; the 64 is
            # folded back via the scalar factors below.
            SC = 64.0
            nc.vector.tensor_scalar_mul(y1h[:], y1h[:], 1.0 / SC)
            # g = factor(s0) (kept off the routing-start critical path)
            factor(s0h[:], g32, "g")
            # f = factor(s1), s1 = s0 + g*y1 = s0 + (64 g)*(y1/64)
            g64 = sm.tile([P, K], F32, tag="g64")
            nc.scalar.mul(g64[:], g32[:], SC)
            outf = sm.tile([P, KO], F32, tag="outf")
            s1f = outf
            nc.vector.tensor_mul(
                s1f[:].rearrange("p (k o) -> p k o", k=K),
                y1h[:].rearrange("p (k o) -> p k o", k=K),
                bcast_o(g64),
            )
            nc.vector.tensor_add(s1f[:], s1f[:], s0h[:])
            # round 2: y2/64 = A (y1/64)
            half_round(y1h, y2h)

            f32_ = sm.tile([P, K], F32, tag="f32_")
            factor(s1f[:], f32_, "f")
            # s2 = s0 + (g+f)*y1 + (f*g)*y2
            #    = s0 + 64(g+f)*(y1/64) + 64(f*g)*(y2/64)
            gf = sm.tile([P, K], F32, tag="gf")
            nc.vector.tensor_add(gf[:], g32[:], f32_[:])
            nc.scalar.mul(gf[:], gf[:], SC)
            fg = sm.tile([P, K], F32, tag="fg")
            nc.vector.tensor_mul(fg[:], f32_[:], g32[:])
            nc.scalar.mul(fg[:], fg[:], SC)
            nc.vector.tensor_mul(
                s2f[:].rearrange("p (k o) -> p k o", k=K),
                y1h[:].rearrange("p (k o) -> p k o", k=K),
                bcast_o(gf),
            )
            nc.vector.tensor_add(s2f[:], s2f[:], s0h[:])
            nc.vector.tensor_mul(
                sqb[:].rearrange("p (k o) -> p k o", k=K),
                y2h[:].rearrange("p (k o) -> p k o", k=K),
                bcast_o(fg),
            )
            nc.vector.tensor_add(s2f[:], s2f[:], sqb[:])
            # out = factor(s2) * s2
            h32 = sm.tile([P, K], F32, tag="h32")
            factor(s2f[:], h32, "h")
            nc.vector.tensor_mul(
                outf[:].rearrange("p (k o) -> p k o", k=K),
                s2f[:].rearrange("p (k o) -> p k o", k=K),
                bcast_o(h32),
            )
        nc.sync.dma_start(out_d, outf[:])

    nc.compile()
    return nc


def _host_prep(x, W):
    """x: [B,R,C,I,D] f32; W: [K,I,D,O] f32 -> per-core Xt + shared W_r.

    Xt[(i%4)*32+d, (i//4)*128+p] = x[p, i, d] (d < 16, pad to 32).
    W_r[(i%4)*32+d, (i//4)*512+k*16+o] = W[k, i, d, o].
    """
    xs = x.reshape(N_CORES, P, I, D)
    a = xs.transpose(0, 2, 3, 1).reshape(N_CORES, 32, 4, D, P)
    ap = np.zeros((N_CORES, 32, 4, D2, P), np.float32)
    ap[:, :, :, 0:D, :] = a
    xt = (
        ap.transpose(0, 2, 3, 1, 4)
        .reshape(N_CORES, 128, 32 * 128)
        .astype(np.float16)
    )
    b = W.transpose(1, 2, 0, 3).reshape(32, 4, D, KO)
    bp = np.zeros((32, 4, D2, KO), np.float32)
    bp[:, :, 0:D, :] = b
    wr = bp.transpose(1, 2, 0, 3).reshape(128, 32 * KO).astype(np.float16)
    return xt, wr


def _get_program():
    global _PROGRAM
    if _PROGRAM is None:
        _PROGRAM = _build_program()
    return _PROGRAM


def kernel(**inputs):
    x = np.ascontiguousarray(np.asarray(inputs["inputs"], dtype=np.float32))
    W = np.ascontiguousarray(np.asarray(inputs["W"], dtype=np.float32))
    assert x.shape == (16, 8, 8, 128, 16) and W.shape == (32, 128, 16, 16)

    from concourse.bass_utils import run_bass_kernel_spmd

    nc = _get_program()
    xt, wr = _host_prep(x, W)
    in_maps = [
        {"xt": np.ascontiguousarray(xt[c]), "wr": wr} for c in range(N_CORES)
    ]
    r = run_bass_kernel_spmd(nc, in_maps, list(range(N_CORES)))
    outs = [r.results[c]["out"].reshape(2, 8, 8, K, O) for c in range(N_CORES)]
    return np.concatenate(outs, axis=0).astype(np.float32)
